# revision 11
# baseline (speedup 1.0000x reference)
"""MultiHeadAttention Trainium2 kernel (8-core SPMD).

Reference semantics (faithful to source, incl. the head/batch-mixing
reshape before the output projection):
    qkv = x @ Wqkv + bqkv ; split into per-head q,k,v (HD=64, H=16)
    scores = q @ k^T / 8 + mask * -1e9 ; attn = softmax(scores)
    values = attn @ v                      # [B,H,S,HD]
    values = values.transpose(1,0,2,3).reshape(B, S, H*HD)
    out = values @ Wo + bo

The transpose+reshape means: for (b, h), out rows
  b' = h // 8, s' = (h % 8) * 256 + b * 128 + i   (i in 0..127)
are  values[b,h].reshape(128, 1024) @ Wo + bo .

Sharding: core c -> batch b = c // 4, heads {4g..4g+3}, g = c % 4.
Each core computes its 4 heads' attention + its 4 output row-blocks.
All big matmuls run in float32r (full-rate PE, ~2e-4 rel err).
"""
import sys
import numpy as np

sys.path.insert(0, "/opt/trn_rl_repo")

B, S, D, H = 2, 2048, 1024, 16
HD = 64
NCORES = 8
HPC = 4          # heads per core
NJ = S // 128    # 16 key tiles
MASK_PE_NUM = 5  # of every 16 mask units, this many go to PE (rest DVE)

_CACHE = {}


def _build_program(stage=4):
    import concourse.bass as bass
    import concourse.bacc as bacc
    import concourse.tile as tile
    from concourse import mybir
    from concourse.masks import make_identity
    from contextlib import ExitStack

    f32 = mybir.dt.float32
    f32r = mybir.dt.float32r
    bf16 = mybir.dt.bfloat16

    nc = bacc.Bacc("TRN2", target_bir_lowering=False, debug=False)

    # ---- DRAM I/O (per-core data) ----
    xt_d = nc.dram_tensor("xt", [D, S], f32r, kind="ExternalInput")          # x[b].T
    wqk_d = nc.dram_tensor("wqk", [D, 512], f32r, kind="ExternalInput")      # [din, q(4x64)|k(4x64)]
    wv_d = nc.dram_tensor("wv", [D, 256], f32r, kind="ExternalInput")        # [din, v(4x64)]
    bqk_d = nc.dram_tensor("bqk", [512], f32, kind="ExternalInput")
    bv_d = nc.dram_tensor("bv", [256], f32r, kind="ExternalInput")
    maskt_d = nc.dram_tensor("maskt", [S, S], bf16, kind="ExternalInput")    # mask[b].T * -8e9
    wo_d = nc.dram_tensor("wo", [D, D], f32r, kind="ExternalInput")
    bo_d = nc.dram_tensor("bo", [D], f32r, kind="ExternalInput")
    out_d = nc.dram_tensor("out", [HPC, 128, D], f32, kind="ExternalOutput")

    xt_r = xt_d[:].rearrange("(t p) s -> p t s", p=128)        # [128, 8, 2048]
    wqk_r = wqk_d[:].rearrange("(t p) m -> p t m", p=128)      # [128, 8, 512]
    wv_r = wv_d[:].rearrange("(t p) m -> p t m", p=128)        # [128, 8, 256]
    bqk_r = bqk_d[:].rearrange("(m p) -> p m", p=128)          # [128, 4]
    maskt_r = maskt_d[:].rearrange("(t p) i -> p t i", p=128)  # [128, 16, 2048]
    wo_r = wo_d[:].rearrange("(c p) n -> p c n", p=128)        # [128, 8, 1024]

    Exp = mybir.ActivationFunctionType.Exp

    with tile.TileContext(nc) as tc, ExitStack() as ctx:
        const = ctx.enter_context(tc.tile_pool(name="const", bufs=1))
        xst = ctx.enter_context(tc.tile_pool(name="xst", bufs=2))
        mst = ctx.enter_context(tc.tile_pool(name="mst", bufs=2))
        est = ctx.enter_context(tc.tile_pool(name="est", bufs=2))
        wrk = ctx.enter_context(tc.tile_pool(name="wrk", bufs=1))
        osb = ctx.enter_context(tc.tile_pool(name="osb", bufs=2))
        psA = ctx.enter_context(tc.tile_pool(name="psA", bufs=2, space="PSUM"))
        psB = ctx.enter_context(tc.tile_pool(name="psB", bufs=2, space="PSUM"))

        # ---- persistent SBUF ----
        wqk_sb = const.tile([128, 8, 512], f32r)
        wv_sb = const.tile([128, 8, 256], f32r)
        wo_sb = const.tile([128, 8, 1024], f32r)
        bqk_sb = const.tile([128, 4], f32)
        bv_sb = const.tile([1, 256], f32r)
        bo_sb = const.tile([1, 1024], f32r)
        ones1 = const.tile([1, 128], f32r)
        ones64 = const.tile([65, 128], f32)
        ones64_col = const.tile([128, NJ * HPC], f32)
        ident = const.tile([128, 128], bf16)
        v_sb = const.tile([128, NJ, HPC, 65], f32r)            # [s%128, s//128, h, hd|1]
        qt_sb = [const.tile([128, S], f32r, tag=f"qt{i}", name=f"qt{i}") for i in range(2)]
        kt_sb = [const.tile([128, S], f32r, tag=f"kt{i}", name=f"kt{i}") for i in range(2)]
        valt = [const.tile([128, S], f32r, tag=f"valt{i}", name=f"valt{i}") for i in range(HPC)]

        nc.sync.dma_start(out=wqk_sb[:], in_=wqk_r)
        nc.sync.dma_start(out=wv_sb[:], in_=wv_r)
        nc.sync.dma_start(out=wo_sb[:], in_=wo_r)
        nc.sync.dma_start(out=bqk_sb[:], in_=bqk_r)
        nc.sync.dma_start(out=bv_sb[:], in_=bv_d[:].unsqueeze(0))
        nc.sync.dma_start(out=bo_sb[:], in_=bo_d[:].unsqueeze(0))
        nc.vector.memset(ones64[:], 1.0)
        nc.vector.memset(ones64_col[:], 1.0)
        nc.vector.tensor_copy(ones1[:], ones64[0:1, :])
        make_identity(nc, ident[:])
        nc.vector.tensor_copy(v_sb[:, :, :, 64:65], ones64_col[:].rearrange('p (t h) -> p t h', t=NJ).unsqueeze(3))

        # ---- Phase 1: QKV projections ----
        for nt in range(4):                      # s-chunks of 512
            xt_t = xst.tile([128, 8, 512], f32r, tag="xt")
            nc.sync.dma_start(out=xt_t[:], in_=xt_r[:, :, nt * 512:(nt + 1) * 512])
            for mt in range(4):                  # q0 q1 k0 k1 chunks of 128 douts
                ps = psA.tile([128, 512], f32, tag="sc")
                for kt in range(8):
                    nc.tensor.matmul(ps[:], wqk_sb[:, kt, mt * 128:(mt + 1) * 128],
                                     xt_t[:, kt, :], start=(kt == 0), stop=(kt == 7))
                dest = qt_sb[mt] if mt < 2 else kt_sb[mt - 2]
                nc.vector.tensor_scalar_add(
                    out=dest[:, nt * 512:(nt + 1) * 512], in0=ps[:],
                    scalar1=bqk_sb[:, mt:mt + 1])
            for sub in range(4):                 # v, natural orientation, s-tiles of 128
                st = nt * 4 + sub
                ps = psB.tile([128, 256], f32, tag="acc")
                nc.tensor.matmul(ps[:], ones1[:], bv_sb[:], start=True, stop=False)
                for kt in range(8):
                    nc.tensor.matmul(ps[:], xt_t[:, kt, sub * 128:(sub + 1) * 128],
                                     wv_sb[:, kt, :], start=False, stop=(kt == 7))
                nc.vector.tensor_copy(
                    v_sb[:, st, :, 0:64],
                    ps[:].rearrange("p (h c) -> p h c", h=HPC))

        # ---- Phase 2: attention ----
        mask_ctr = 0
        for ihalf in range(2 if stage >= 2 else 0):
            i0 = ihalf * 1024
            for pr in range(2):
                acc = [psB.tile([128, 1024], f32, tag="acc", name=f"acc{ihalf}{pr}{_}") for _ in range(2)]
                for j in range(NJ):
                    mt_t = mst.tile([128, 1024], bf16, tag="mk")
                    nc.sync.dma_start(out=mt_t[:],
                                      in_=maskt_r[:, j, i0:i0 + 1024])
                    for h2 in range(2):
                        hsl = slice(h2 * 64, (h2 + 1) * 64)
                        sc = psA.tile([128, 1024], f32, tag="sc")
                        on_pe = (mask_ctr % 16) < MASK_PE_NUM
                        mask_ctr += 1
                        for n2 in range(2):
                            nsl = slice(n2 * 512, (n2 + 1) * 512)
                            nc.tensor.matmul(
                                sc[:, nsl],
                                kt_sb[pr][hsl, j * 128:(j + 1) * 128],
                                qt_sb[pr][hsl, i0 + n2 * 512:i0 + (n2 + 1) * 512],
                                start=True, stop=not on_pe,
                                tile_position=(h2 * 64, 0))
                            if on_pe:
                                nc.tensor.matmul(sc[:, nsl], ident[:], mt_t[:, nsl],
                                                 start=False, stop=True)
                        if not on_pe:
                            nc.vector.tensor_add(sc[:], sc[:], mt_t[:])
                        ex = est.tile([128, 1024], f32r, tag="ex")
                        nc.scalar.activation(ex[:], sc[:], Exp, scale=0.125)
                        for n2 in range(2):
                            nsl = slice(n2 * 512, (n2 + 1) * 512)
                            nc.tensor.matmul(acc[h2][0:65, nsl],
                                             v_sb[:, j, pr * 2 + h2, :], ex[:, nsl],
                                             start=(j == 0), stop=(j == NJ - 1))
                for h2 in range(2 if stage >= 3 else 0):
                    h = pr * 2 + h2
                    drow = wrk.tile([65, 1024], f32, tag="drow")
                    nc.vector.tensor_copy(drow[64:65, :], acc[h2][64:65, :])
                    dps = psA.tile([64, 1024], f32, tag="sc", name="dps")
                    for n2 in range(2):
                        nsl = slice(n2 * 512, (n2 + 1) * 512)
                        nc.tensor.matmul(dps[:, nsl], ones64[64:65, 0:64],
                                         drow[64:65, nsl], start=True, stop=True,
                                         tile_position=(64, 0))
                    recb = wrk.tile([64, 1024], f32, tag="recb")
                    scr = wrk.tile([64, 1024], f32, tag="scr")
                    nc.vector.reciprocal_approx_accurate(
                        out=recb[:], in_=dps[:], scratch=scr[:])
                    nc.vector.tensor_mul(valt[h][0:64, i0:i0 + 1024],
                                         acc[h2][0:64, :], recb[:])
                    if ihalf == 1:
                        nc.sync.dma_start(out=valt[h][64:128, :],
                                          in_=valt[h][0:64, :])

        # ---- Phase 3: output projection ----
        for h in range(HPC if stage >= 4 else 0):
            poe = psB.tile([128, 1024], f32, tag="acc", name=f"poe{h}")
            poo = psA.tile([128, 1024], f32, tag="sc", name=f"poo{h}")
            for n2 in range(2):
                nsl = slice(n2 * 512, (n2 + 1) * 512)
                nc.tensor.matmul(poe[:, nsl], ones1[:], bo_sb[:, nsl],
                                 start=True, stop=False)
                for idx, j in enumerate(range(0, NJ, 2)):
                    nc.tensor.matmul(
                        poe[:, nsl], valt[h][0:64, j::16],
                        wo_sb[0:64, j // 2, nsl],
                        start=False, stop=(idx == 7))
                for idx, j in enumerate(range(1, NJ, 2)):
                    nc.tensor.matmul(
                        poo[:, nsl], valt[h][64:128, j::16],
                        wo_sb[64:128, j // 2, nsl],
                        start=(idx == 0), stop=(idx == 7),
                        tile_position=(64, 0))
            ot = osb.tile([128, 1024], f32, tag="ot")
            nc.scalar.copy(ot[:], poe[:])
            nc.vector.tensor_add(ot[:], ot[:], poo[:])
            nc.sync.dma_start(out=out_d[h], in_=ot[:])

        if stage < 4:
            for h in range(HPC):
                ot = osb.tile([128, 1024], f32, tag="ot")
                nc.vector.memset(ot[:], 0.0)
                nc.sync.dma_start(out=out_d[h], in_=ot[:])
    nc.finalize()
    return nc


def _get_program():
    import os
    stage = int(os.environ.get("KSTAGE", "4"))
    key = f"nc{stage}"
    if key not in _CACHE:
        _CACHE[key] = _build_program(stage)
    return _CACHE[key]


def _prep_inputs(x, mask, Wqkv, bqkv, Wo, bo):
    import ml_dtypes
    xT = np.ascontiguousarray(np.transpose(x, (0, 2, 1)), dtype=np.float32)
    maskT = np.ascontiguousarray(np.transpose(mask, (0, 2, 1)), dtype=np.float32)
    maskT = (maskT * np.float32(-8e9)).astype(ml_dtypes.bfloat16)
    Wqkv = np.asarray(Wqkv, np.float32)
    bqkv = np.asarray(bqkv, np.float32)
    Wo = np.ascontiguousarray(Wo, np.float32)
    bo = np.asarray(bo, np.float32)
    in_maps = []
    for c in range(NCORES):
        b, g = c // 4, c % 4
        hs = [4 * g + i for i in range(HPC)]
        qcols = np.concatenate([np.arange(h * HD, (h + 1) * HD) for h in hs])
        wqk = np.concatenate([Wqkv[:, qcols], Wqkv[:, D + qcols]], axis=1)
        wv = Wqkv[:, 2 * D + qcols]
        bqk = np.concatenate([bqkv[qcols], bqkv[D + qcols]])
        bv = bqkv[2 * D + qcols]
        in_maps.append({
            "xt": xT[b],
            "wqk": np.ascontiguousarray(wqk),
            "wv": np.ascontiguousarray(wv),
            "bqk": np.ascontiguousarray(bqk),
            "bv": np.ascontiguousarray(bv),
            "maskt": maskT[b],
            "wo": Wo,
            "bo": bo,
        })
    return in_maps


def _scatter_output(results):
    out = np.empty((B, S, D), np.float32)
    for c in range(NCORES):
        b, g = c // 4, c % 4
        blk = results[c]["out"]          # [4, 128, 1024]
        for i in range(HPC):
            h = 4 * g + i
            bp = h // 8
            sb = (h % 8) * 256 + b * 128
            out[bp, sb:sb + 128, :] = blk[i]
    return out


def kernel(x, mask, Wqkv, bqkv, Wo, bo, _trace=False):
    from concourse.bass_utils import run_bass_kernel_spmd
    nc = _get_program()
    in_maps = _prep_inputs(x, mask, Wqkv, bqkv, Wo, bo)
    res = run_bass_kernel_spmd(nc, in_maps, core_ids=list(range(NCORES)),
                               trace=_trace)
    out = _scatter_output(res.results)
    if _trace:
        return out, res
    return out


# revision 13
# speedup vs baseline: 1.0882x; 1.0882x over previous
"""MultiHeadAttention Trainium2 kernel (8-core SPMD).

Reference semantics (faithful to source, incl. the head/batch-mixing
reshape before the output projection):
    qkv = x @ Wqkv + bqkv ; split into per-head q,k,v (HD=64, H=16)
    scores = q @ k^T / 8 + mask * -1e9 ; attn = softmax(scores)
    values = attn @ v                      # [B,H,S,HD]
    values = values.transpose(1,0,2,3).reshape(B, S, H*HD)
    out = values @ Wo + bo

The transpose+reshape means: for (b, h), out rows
  b' = h // 8, s' = (h % 8) * 256 + b * 128 + i   (i in 0..127)
are  values[b,h].reshape(128, 1024) @ Wo + bo .

Sharding: core c -> batch b = c // 4, heads {4g..4g+3}, g = c % 4.
Each core computes its 4 heads' attention + its 4 output row-blocks.
All big matmuls run in float32r (full-rate PE, ~2e-4 rel err).
"""
import sys
import numpy as np

sys.path.insert(0, "/opt/trn_rl_repo")

B, S, D, H = 2, 2048, 1024, 16
HD = 64
NCORES = 8
HPC = 4          # heads per core
NJ = S // 128    # 16 key tiles
MASK_DVE_NUM = 9  # of every 16 keep-mult units, this many on DVE (rest GPSIMD)

_CACHE = {}


def _build_program(stage=4):
    import concourse.bass as bass
    import concourse.bacc as bacc
    import concourse.tile as tile
    from concourse import mybir
    from contextlib import ExitStack

    f32 = mybir.dt.float32
    f32r = mybir.dt.float32r
    bf16 = mybir.dt.bfloat16

    nc = bacc.Bacc("TRN2", target_bir_lowering=False, debug=False)

    # ---- DRAM I/O (per-core data) ----
    xt_d = nc.dram_tensor("xt", [D, S], f32r, kind="ExternalInput")          # x[b].T
    wqk_d = nc.dram_tensor("wqk", [D, 512], f32r, kind="ExternalInput")      # [din, q(4x64)|k(4x64)]
    wv_d = nc.dram_tensor("wv", [D, 256], f32r, kind="ExternalInput")        # [din, v(4x64)]
    bqk_d = nc.dram_tensor("bqk", [512], f32, kind="ExternalInput")
    bv_d = nc.dram_tensor("bv", [256], f32r, kind="ExternalInput")
    maskt_d = nc.dram_tensor("maskt", [S, S], bf16, kind="ExternalInput")    # keep = 1-mask[b].T
    wo_d = nc.dram_tensor("wo", [D, D], f32r, kind="ExternalInput")
    bo_d = nc.dram_tensor("bo", [D], f32r, kind="ExternalInput")
    out_d = nc.dram_tensor("out", [HPC, 128, D], f32, kind="ExternalOutput")

    xt_r = xt_d[:].rearrange("(t p) s -> p t s", p=128)        # [128, 8, 2048]
    wqk_r = wqk_d[:].rearrange("(t p) m -> p t m", p=128)      # [128, 8, 512]
    wv_r = wv_d[:].rearrange("(t p) m -> p t m", p=128)        # [128, 8, 256]
    bqk_r = bqk_d[:].rearrange("(m p) -> p m", p=128)          # [128, 4]
    maskt_r = maskt_d[:].rearrange("(t p) i -> p t i", p=128)  # [128, 16, 2048]
    wo_r = wo_d[:].rearrange("(c p) n -> p c n", p=128)        # [128, 8, 1024]

    Exp = mybir.ActivationFunctionType.Exp

    with tile.TileContext(nc) as tc, ExitStack() as ctx:
        const = ctx.enter_context(tc.tile_pool(name="const", bufs=1))
        xst = ctx.enter_context(tc.tile_pool(name="xst", bufs=2))
        mst = ctx.enter_context(tc.tile_pool(name="mst", bufs=2))
        est = ctx.enter_context(tc.tile_pool(name="est", bufs=2))
        wrk = ctx.enter_context(tc.tile_pool(name="wrk", bufs=1))
        osb = ctx.enter_context(tc.tile_pool(name="osb", bufs=2))
        psA = ctx.enter_context(tc.tile_pool(name="psA", bufs=2, space="PSUM"))
        psB = ctx.enter_context(tc.tile_pool(name="psB", bufs=2, space="PSUM"))

        # ---- persistent SBUF ----
        wqk_sb = const.tile([128, 8, 512], f32r)
        wv_sb = const.tile([128, 8, 256], f32r)
        wo_sb = const.tile([128, 8, 1024], f32r)
        bqk_sb = const.tile([128, 4], f32)
        bv_sb = const.tile([1, 256], f32r)
        bo_sb = const.tile([1, 1024], f32r)
        ones1 = const.tile([1, 128], f32r)
        ones64 = const.tile([65, 128], f32)
        ones64_col = const.tile([128, NJ * HPC], f32)
        v_sb = const.tile([128, NJ, HPC, 65], f32r)            # [s%128, s//128, h, hd|1]
        qt_sb = [const.tile([128, S], f32r, tag=f"qt{i}", name=f"qt{i}") for i in range(2)]
        kt_sb = [const.tile([128, S], f32r, tag=f"kt{i}", name=f"kt{i}") for i in range(2)]
        valt = [const.tile([128, S], f32r, tag=f"valt{i}", name=f"valt{i}") for i in range(HPC)]

        nc.sync.dma_start(out=wqk_sb[:], in_=wqk_r)
        nc.sync.dma_start(out=wv_sb[:], in_=wv_r)
        nc.sync.dma_start(out=wo_sb[:], in_=wo_r)
        nc.sync.dma_start(out=bqk_sb[:], in_=bqk_r)
        nc.sync.dma_start(out=bv_sb[:], in_=bv_d[:].unsqueeze(0))
        nc.sync.dma_start(out=bo_sb[:], in_=bo_d[:].unsqueeze(0))
        nc.vector.memset(ones64[:], 1.0)
        nc.vector.memset(ones64_col[:], 1.0)
        nc.vector.tensor_copy(ones1[:], ones64[0:1, :])
        nc.vector.tensor_copy(v_sb[:, :, :, 64:65], ones64_col[:].rearrange('p (t h) -> p t h', t=NJ).unsqueeze(3))

        # ---- Phase 1: QKV projections ----
        for nt in range(4):                      # s-chunks of 512
            xt_t = xst.tile([128, 8, 512], f32r, tag="xt")
            nc.sync.dma_start(out=xt_t[:], in_=xt_r[:, :, nt * 512:(nt + 1) * 512])
            for mt in range(4):                  # q0 q1 k0 k1 chunks of 128 douts
                ps = psA.tile([128, 512], f32, tag="sc")
                for kt in range(8):
                    nc.tensor.matmul(ps[:], wqk_sb[:, kt, mt * 128:(mt + 1) * 128],
                                     xt_t[:, kt, :], start=(kt == 0), stop=(kt == 7))
                dest = qt_sb[mt] if mt < 2 else kt_sb[mt - 2]
                nc.vector.tensor_scalar_add(
                    out=dest[:, nt * 512:(nt + 1) * 512], in0=ps[:],
                    scalar1=bqk_sb[:, mt:mt + 1])
            for sub in range(4):                 # v, natural orientation, s-tiles of 128
                st = nt * 4 + sub
                ps = psB.tile([128, 256], f32, tag="acc")
                nc.tensor.matmul(ps[:], ones1[:], bv_sb[:], start=True, stop=False)
                for kt in range(8):
                    nc.tensor.matmul(ps[:], xt_t[:, kt, sub * 128:(sub + 1) * 128],
                                     wv_sb[:, kt, :], start=False, stop=(kt == 7))
                nc.vector.tensor_copy(
                    v_sb[:, st, :, 0:64],
                    ps[:].rearrange("p (h c) -> p h c", h=HPC))

        # ---- Phase 2: attention ----
        mask_ctr = 0
        for ihalf in range(2 if stage >= 2 else 0):
            i0 = ihalf * 1024
            for pr in range(2):
                acc = [psB.tile([128, 1024], f32, tag="acc", name=f"acc{ihalf}{pr}{_}") for _ in range(2)]
                for j in range(NJ):
                    mt_t = mst.tile([128, 1024], bf16, tag="mk")
                    nc.sync.dma_start(out=mt_t[:],
                                      in_=maskt_r[:, j, i0:i0 + 1024])
                    for h2 in range(2):
                        hsl = slice(h2 * 64, (h2 + 1) * 64)
                        sc = psA.tile([128, 1024], f32, tag="sc")
                        for n2 in range(2):
                            nsl = slice(n2 * 512, (n2 + 1) * 512)
                            nc.tensor.matmul(
                                sc[:, nsl],
                                kt_sb[pr][hsl, j * 128:(j + 1) * 128],
                                qt_sb[pr][hsl, i0 + n2 * 512:i0 + (n2 + 1) * 512],
                                start=True, stop=True,
                                tile_position=(h2 * 64, 0))
                        ex = est.tile([128, 1024], f32r, tag="ex")
                        nc.scalar.activation(ex[:], sc[:], Exp, scale=0.125)
                        on_dve = (mask_ctr % 16) < MASK_DVE_NUM
                        mask_ctr += 1
                        eng = nc.vector if on_dve else nc.gpsimd
                        eng.tensor_mul(ex[:], ex[:], mt_t[:])
                        for n2 in range(2):
                            nsl = slice(n2 * 512, (n2 + 1) * 512)
                            nc.tensor.matmul(acc[h2][0:65, nsl],
                                             v_sb[:, j, pr * 2 + h2, :], ex[:, nsl],
                                             start=(j == 0), stop=(j == NJ - 1))
                for h2 in range(2 if stage >= 3 else 0):
                    h = pr * 2 + h2
                    drow = wrk.tile([65, 1024], f32, tag="drow")
                    nc.vector.tensor_copy(drow[64:65, :], acc[h2][64:65, :])
                    dps = psA.tile([64, 1024], f32, tag="sc", name="dps")
                    for n2 in range(2):
                        nsl = slice(n2 * 512, (n2 + 1) * 512)
                        nc.tensor.matmul(dps[:, nsl], ones64[64:65, 0:64],
                                         drow[64:65, nsl], start=True, stop=True,
                                         tile_position=(64, 0))
                    recb = wrk.tile([64, 1024], f32, tag="recb")
                    scr = wrk.tile([64, 1024], f32, tag="scr")
                    nc.vector.reciprocal_approx_accurate(
                        out=recb[:], in_=dps[:], scratch=scr[:])
                    nc.vector.tensor_mul(valt[h][0:64, i0:i0 + 1024],
                                         acc[h2][0:64, :], recb[:])
                    if ihalf == 1:
                        nc.sync.dma_start(out=valt[h][64:128, :],
                                          in_=valt[h][0:64, :])

        # ---- Phase 3: output projection ----
        for h in range(HPC if stage >= 4 else 0):
            poe = psB.tile([128, 1024], f32, tag="acc", name=f"poe{h}")
            poo = psA.tile([128, 1024], f32, tag="sc", name=f"poo{h}")
            for n2 in range(2):
                nsl = slice(n2 * 512, (n2 + 1) * 512)
                nc.tensor.matmul(poe[:, nsl], ones1[:], bo_sb[:, nsl],
                                 start=True, stop=False)
                for idx, j in enumerate(range(0, NJ, 2)):
                    nc.tensor.matmul(
                        poe[:, nsl], valt[h][0:64, j::16],
                        wo_sb[0:64, j // 2, nsl],
                        start=False, stop=(idx == 7))
                for idx, j in enumerate(range(1, NJ, 2)):
                    nc.tensor.matmul(
                        poo[:, nsl], valt[h][64:128, j::16],
                        wo_sb[64:128, j // 2, nsl],
                        start=(idx == 0), stop=(idx == 7),
                        tile_position=(64, 0))
            ot = osb.tile([128, 1024], f32, tag="ot")
            nc.scalar.copy(ot[:], poe[:])
            nc.vector.tensor_add(ot[:], ot[:], poo[:])
            nc.sync.dma_start(out=out_d[h], in_=ot[:])

        if stage < 4:
            for h in range(HPC):
                ot = osb.tile([128, 1024], f32, tag="ot")
                nc.vector.memset(ot[:], 0.0)
                nc.sync.dma_start(out=out_d[h], in_=ot[:])
    nc.finalize()
    return nc


def _get_program():
    import os
    stage = int(os.environ.get("KSTAGE", "4"))
    key = f"nc{stage}"
    if key not in _CACHE:
        _CACHE[key] = _build_program(stage)
    return _CACHE[key]


def _prep_inputs(x, mask, Wqkv, bqkv, Wo, bo):
    import ml_dtypes
    xT = np.ascontiguousarray(np.transpose(x, (0, 2, 1)), dtype=np.float32)
    maskT = np.ascontiguousarray(np.transpose(mask, (0, 2, 1)), dtype=np.float32)
    maskT = (np.float32(1.0) - maskT).astype(ml_dtypes.bfloat16)  # keep mask
    Wqkv = np.asarray(Wqkv, np.float32)
    bqkv = np.asarray(bqkv, np.float32)
    Wo = np.ascontiguousarray(Wo, np.float32)
    bo = np.asarray(bo, np.float32)
    in_maps = []
    for c in range(NCORES):
        b, g = c // 4, c % 4
        hs = [4 * g + i for i in range(HPC)]
        qcols = np.concatenate([np.arange(h * HD, (h + 1) * HD) for h in hs])
        wqk = np.concatenate([Wqkv[:, qcols], Wqkv[:, D + qcols]], axis=1)
        wv = Wqkv[:, 2 * D + qcols]
        bqk = np.concatenate([bqkv[qcols], bqkv[D + qcols]])
        bv = bqkv[2 * D + qcols]
        in_maps.append({
            "xt": xT[b],
            "wqk": np.ascontiguousarray(wqk),
            "wv": np.ascontiguousarray(wv),
            "bqk": np.ascontiguousarray(bqk),
            "bv": np.ascontiguousarray(bv),
            "maskt": maskT[b],
            "wo": Wo,
            "bo": bo,
        })
    return in_maps


def _scatter_output(results):
    out = np.empty((B, S, D), np.float32)
    for c in range(NCORES):
        b, g = c // 4, c % 4
        blk = results[c]["out"]          # [4, 128, 1024]
        for i in range(HPC):
            h = 4 * g + i
            bp = h // 8
            sb = (h % 8) * 256 + b * 128
            out[bp, sb:sb + 128, :] = blk[i]
    return out


def kernel(x, mask, Wqkv, bqkv, Wo, bo, _trace=False):
    from concourse.bass_utils import run_bass_kernel_spmd
    nc = _get_program()
    in_maps = _prep_inputs(x, mask, Wqkv, bqkv, Wo, bo)
    res = run_bass_kernel_spmd(nc, in_maps, core_ids=list(range(NCORES)),
                               trace=_trace)
    out = _scatter_output(res.results)
    if _trace:
        return out, res
    return out
